# revision 7
# baseline (speedup 1.0000x reference)
"""Capsule-routing kernel for 8 Trainium2 NeuronCores.

Problem: B=64, I=256 input capsules, Din=256, N=32 capsules x D=64 dims,
ROUTINGS=2 dynamic-routing iterations.

    hat = einsum('bid,ido->bio', x, kernel)          # 256 independent
                                                     # [64,256]@[256,2048] matmuls
    r0:  c uniform (softmax of zeros over N) -> o0 = squash(sum_i hat / 32)
         b1 = einsum('bnd,bnid->bni', o0, hat)
    r1:  c1 = softmax(b1, axis=N) -> o = squash(einsum('bni,bnid->bnd', c1, hat))

Sharding: over I (32 input capsules per core).  The softmax is over N
(not sharded) per (b, i), so routing is local per core except for the
two capsule-sum reductions, which are tiny ([64, 2048] = 0.5 MB):
  - sum_i hat   -> AllReduce on device (needed for o0 mid-kernel)
  - final sum_i c1*hat -> partial per core, summed + squashed on host.

Per-core device work: read a 67 MB kernel shard (DMA bound, ~190 us),
keep the local hat [64, 32, 2048] = 16.8 MB resident in SBUF (fp32),
then a DVE routing tail.
"""

import os
import sys

import numpy as np

for _p in ("/opt/trn_rl_repo", "/root/.axon_site/_ro/trn_rl_repo"):
    if os.path.isdir(_p) and _p not in sys.path:
        sys.path.insert(0, _p)

# ---------------- problem constants (hardcoded per contract) ----------------
B = 64          # batch
I = 256         # input capsules
DIN = 256       # input capsule dim
N = 32          # output capsules
D = 64          # output capsule dim
NO = N * D      # 2048
NCORES = 8
ILOC = I // NCORES   # 32 input capsules per core
NPAIR = ILOC // 2    # 16 i-pairs (two i packed on the 128 partitions)
EPS = 1e-7
KCH = DIN // 128     # 2 contraction chunks of 128
NCH = NO // 512      # 4 psum-bank chunks of 512

F32 = None  # set after imports


def _imports():
    import concourse.bass as bass
    import concourse.tile as tile
    import concourse.mybir as mybir
    from concourse import bacc
    return bass, tile, mybir, bacc


def build_kernel(tc, outs, ins, num_cores=NCORES, hat_bf16=False):
    """Emit the per-core program.  ins = {'xt': AP, 'kn': AP},
    outs = {'p1p': AP}.

    xt: [128, ILOC*KCH*B]  f32   xt[p, (i*2+ch)*64 + b] = x[b, i, ch*128+p]
    kn: [ILOC, KCH, 128, NO] f32 kn[i, ch, p, o] = kernel[i, ch*128+p, o]
    p1p (out): [B, NO] f32       partial sum_i_local c1*hat
    """
    bass, tile, mybir, bacc = _imports()
    f32 = mybir.dt.float32
    bf16 = mybir.dt.bfloat16
    hat_dt = bf16 if hat_bf16 else f32
    nc = tc.nc
    ts = bass.ts

    xt = ins["xt"]
    kn = ins["kn"]
    p1p = outs["p1p"]

    with (
        tc.tile_pool(name="hat", bufs=1) as hat_pool,
        tc.tile_pool(name="xt", bufs=1) as xt_pool,
        tc.tile_pool(name="kn", bufs=3) as kn_pool,
        tc.tile_pool(name="acc", bufs=1) as acc_pool,
        tc.tile_pool(name="tmp", bufs=1) as tmp_pool,
        tc.tile_pool(name="small", bufs=1) as small_pool,
        tc.tile_pool(name="psum", bufs=2, space="PSUM") as psum_pool,
        tc.tile_pool(name="dram", bufs=1, space="DRAM") as dram_pool,
    ):
        # ---------- phase 1: matmuls, hat residency, p0 accumulation ----------
        xt_sb = xt_pool.tile([128, ILOC * KCH * B], f32)
        nc.sync.dma_start(xt_sb[:], xt[:])

        hat_all = hat_pool.tile([128, NPAIR, NO], hat_dt)
        p0acc = acc_pool.tile([128, NO], f32, tag="acc")

        for t in range(NPAIR):
            # kernel chunks for the two capsules of this pair
            kt = []
            for h in range(2):
                for ch in range(KCH):
                    k_sb = kn_pool.tile([128, NO], f32, tag="kn")
                    nc.sync.dma_start(k_sb[:], kn[2 * t + h, ch])
                    kt.append(k_sb)

            ps = psum_pool.tile([128, NO], f32)
            for h in range(2):
                pslice = ps[h * 64:(h + 1) * 64]
                tpos = (0, 64) if h == 1 else None
                for nch in range(NCH):
                    for ch in range(KCH):
                        lhsT = xt_sb[:, ts(4 * t + 2 * h + ch, B)]
                        rhs = kt[2 * h + ch][:, ts(nch, 512)]
                        nc.tensor.matmul(
                            pslice[:, ts(nch, 512)],
                            lhsT,
                            rhs,
                            start=(ch == 0),
                            stop=(ch == KCH - 1),
                            tile_position=tpos,
                        )

            # hat residency copy (ACT) + p0 accumulation (DVE)
            nc.scalar.copy(hat_all[:, t, :], ps[:])
            if t == 0:
                nc.vector.tensor_copy(p0acc[:], ps[:])
            else:
                nc.vector.tensor_add(p0acc[:], p0acc[:], ps[:])

        # ---------- phase 2: fold + AllReduce of p0 ----------
        # (walrus rejects SB+SB tensor_tensor at different base partitions,
        #  so bounce the upper half down via SBUF->SBUF DMA first)
        p0f = small_pool.tile([64, NO], f32, tag="p0f")
        fhi = tmp_pool.tile([64, NO], f32, tag="tmp")
        nc.sync.dma_start(fhi[:], p0acc[64:128, :])
        nc.vector.tensor_add(p0f[:], p0acc[0:64, :], fhi[:])

        ar_in = dram_pool.tile([64, NO], f32, tag="arin")
        ar_out = dram_pool.tile([64, NO], f32, tag="arout", addr_space="Shared")
        nc.sync.dma_start(ar_in[:], p0f[:])
        groups = [list(range(num_cores))]
        nc.gpsimd.collective_compute(
            "AllReduce",
            mybir.AluOpType.add,
            replica_groups=groups,
            ins=[ar_in[:].opt()],
            outs=[ar_out[:].opt()],
        )
        s0 = small_pool.tile([64, NO], f32, tag="p0f")
        nc.sync.dma_start(s0[:], ar_out[:])

        # ---------- phase 3: o0 = squash(s0/32) ----------
        # sq[b,n] = sum_d (s0/32)^2 ;  scale = sqrt(sq+EPS)/(1+sq+EPS)
        sqf = tmp_pool.tile([128, NO], f32, tag="tmp")
        nc.scalar.activation(sqf[0:64, :], s0[:],
                             mybir.ActivationFunctionType.Square,
                             scale=1.0 / 32.0)
        sq = small_pool.tile([64, N], f32, tag="sq")
        nc.vector.reduce_sum(sq[:], sqf[0:64].rearrange("p (n d) -> p n d", d=D),
                             axis=mybir.AxisListType.X)
        sqe = small_pool.tile([64, N], f32, tag="sqe")
        nc.vector.tensor_scalar_add(sqe[:], sq[:], EPS)
        rt = small_pool.tile([64, N], f32, tag="rt")
        nc.scalar.activation(rt[:], sqe[:], mybir.ActivationFunctionType.Sqrt)
        den = small_pool.tile([64, N], f32, tag="den")
        nc.vector.tensor_scalar_add(den[:], sqe[:], 1.0)
        rec = small_pool.tile([64, N], f32, tag="rec")
        nc.vector.reciprocal(rec[:], den[:])
        sc = small_pool.tile([64, N], f32, tag="sc")
        nc.vector.tensor_mul(sc[:], rt[:], rec[:])

        # o0 = (s0/32) * sc  (broadcast over d), into both partition halves
        o0d = small_pool.tile([128, NO], hat_dt, tag="o0d")
        sc_b = sc[:].unsqueeze(2).broadcast_to([64, N, D])
        nc.vector.scalar_tensor_tensor(
            o0d[0:64].rearrange("p (n d) -> p n d", d=D),
            s0[:].rearrange("p (n d) -> p n d", d=D),
            1.0 / 32.0,
            sc_b,
            op0=mybir.AluOpType.mult,
            op1=mybir.AluOpType.mult,
        )
        # duplicate to partitions 64-127 (SBUF->SBUF DMA crosses partitions)
        nc.sync.dma_start(o0d[64:128, :], o0d[0:64, :])

        # ---------- phase 4: b1 = sum_d hat*o0 ; c1 = softmax_n(b1) ----------
        b1 = small_pool.tile([128, NPAIR, N], f32, tag="b1")
        for t in range(NPAIR):
            tmp = tmp_pool.tile([128, NO], hat_dt, tag="tmp")
            nc.vector.tensor_mul(tmp[:], hat_all[:, t, :], o0d[:])
            nc.vector.reduce_sum(b1[:, t, :],
                                 tmp[:].rearrange("p (n d) -> p n d", d=D),
                                 axis=mybir.AxisListType.X)

        e1 = tmp_pool.tile([128, NPAIR, N], f32, tag="tmp")
        nc.scalar.activation(e1[:], b1[:], mybir.ActivationFunctionType.Exp)
        z = small_pool.tile([128, NPAIR], f32, tag="z")
        nc.vector.reduce_sum(z[:], e1[:], axis=mybir.AxisListType.X)
        zr = small_pool.tile([128, NPAIR], f32, tag="zr")
        nc.vector.reciprocal(zr[:], z[:])
        c1 = small_pool.tile([128, NPAIR, N], hat_dt, tag="c1")
        nc.vector.tensor_mul(c1[:], e1[:],
                             zr[:].unsqueeze(2).broadcast_to([128, NPAIR, N]))

        # ---------- phase 5: p1 = sum_i c1*hat (local partial) ----------
        p1acc = acc_pool.tile([128, NO], f32, tag="acc")
        for t in range(NPAIR):
            tmp = tmp_pool.tile([128, NO], hat_dt, tag="tmp")
            c1_b = c1[:, t, :].unsqueeze(2).broadcast_to([128, N, D])
            nc.vector.tensor_mul(tmp[:].rearrange("p (n d) -> p n d", d=D),
                                 hat_all[:, t, :].rearrange("p (n d) -> p n d", d=D),
                                 c1_b)
            if t == 0:
                nc.vector.tensor_copy(p1acc[:], tmp[:])
            else:
                nc.vector.tensor_add(p1acc[:], p1acc[:], tmp[:])

        p1f = small_pool.tile([64, NO], f32, tag="p0f")
        fhi2 = tmp_pool.tile([64, NO], f32, tag="tmp")
        nc.sync.dma_start(fhi2[:], p1acc[64:128, :])
        nc.vector.tensor_add(p1f[:], p1acc[0:64, :], fhi2[:])
        nc.sync.dma_start(p1p[:], p1f[:])


# ---------------- host-side wrapper ----------------

_CACHE = {}


def _build_program(num_cores=NCORES, hat_bf16=False):
    key = (num_cores, hat_bf16)
    if key in _CACHE:
        return _CACHE[key]
    bass, tile, mybir, bacc = _imports()
    f32 = mybir.dt.float32
    nc = bacc.Bacc("TRN2", target_bir_lowering=False, debug=False,
                   num_devices=num_cores)
    xt_t = nc.dram_tensor("xt", [128, ILOC * KCH * B], f32, kind="ExternalInput")
    kn_t = nc.dram_tensor("kn", [ILOC, KCH, 128, NO], f32, kind="ExternalInput")
    out_t = nc.dram_tensor("p1p", [B, NO], f32, kind="ExternalOutput")
    with tile.TileContext(nc) as tc:
        build_kernel(tc, {"p1p": out_t.ap()}, {"xt": xt_t.ap(), "kn": kn_t.ap()},
                     num_cores=num_cores, hat_bf16=hat_bf16)
    nc.compile()
    _CACHE[key] = nc
    return nc


def make_core_inputs(x, kernel, core):
    """Per-core input tensors (numpy, host-side layout prep)."""
    sl = slice(core * ILOC, (core + 1) * ILOC)
    # xt[p, (i*KCH+ch)*B + b] = x[b, i_loc, ch*128+p]
    xs = np.ascontiguousarray(x[:, sl, :])                 # [B, ILOC, DIN]
    xt = xs.reshape(B, ILOC, KCH, 128).transpose(3, 1, 2, 0)  # [128, ILOC, KCH, B]
    xt = np.ascontiguousarray(xt).reshape(128, ILOC * KCH * B)
    kc = np.ascontiguousarray(kernel[sl]).reshape(ILOC, KCH, 128, NO)
    return {"xt": xt, "kn": kc}


def finish_host(partials):
    """Sum per-core p1 partials and apply squash -> [B, N, D] f32."""
    p1 = np.zeros((B, NO), np.float32)
    for p in partials:
        p1 += p
    v = p1.reshape(B, N, D)
    sq = np.sum(np.square(v), axis=-1, keepdims=True) + EPS
    scale = np.sqrt(sq) / (1.0 + sq)
    return (scale * v).astype(np.float32)


def run_on_cores(x, kernel, trace=False, hat_bf16=False, trace_kwargs=None):
    from concourse.bass_utils import run_bass_kernel_spmd
    nc = _build_program(NCORES, hat_bf16=hat_bf16)
    in_maps = [make_core_inputs(x, kernel, c) for c in range(NCORES)]
    res = run_bass_kernel_spmd(
        nc, in_maps, core_ids=list(range(NCORES)),
        trace=trace, **(trace_kwargs or {}),
    )
    return res


def kernel(x, kernel):
    x = np.asarray(x, np.float32)
    kernel = np.asarray(kernel, np.float32)
    res = run_on_cores(x, kernel)
    return finish_host([res.results[c]["p1p"] for c in range(NCORES)])
